# revision 4
# baseline (speedup 1.0000x reference)
"""EntNet Trainium2 kernel, v3.

B=32, S=256, L=64, D=100, M=20. Data-parallel over batch: 8 cores x B_loc=4.
State rows m-major (row = m*4 + b), BM=80 rows per core.

Scan state is UNNORMALIZED P (fp16) with alpha = 1/||P|| per row
(true memory = alpha * P). Per step t (anchor: e_t = exp(-z_t) ready):

  Pool: d = e + 1
  DVE : g = 1/d                  (gate = sigmoid(z) = 1/(1+exp(-z)))
  DVE : P' = cfull*g + mem       (single fused stt, fp16)
  DVE : ssq = sum P'^2           (stt bypass-mult with accum_out)
  DVE : zp  = sum P'*(-s_{t+1})  (stt-dot; sbc holds -enc)
  PE  : transpose(P') -> fp16 PSUM
  ACT : Ln(ssq); alpha' = Exp(-0.5*ln); e' = Exp(zp, scale=alpha', bias=nkg)
  DVE : pT = copy(transpose PSUM) -> fp16 SBUF
  PE  : candU' = pT^T @ U^T      (PSUM f32)
  DVE : cfull' = candU'*alpha' + cws_{t+1}  (fp16)
  Pool: mem' = P' * alpha'       (fp16)

Encode streams x as fp16 (host-cast, halves HBM); word-sum tree on DVE
(fp16 2x mode); kg/cws builds on Pool; per-engine generators are pulled
at fixed slots in the step loop so in-order queues never stall the chain.
"""

import os
from contextlib import ExitStack

import numpy as np

B, S, L, D, M = 32, 256, 64, 100, 20
NCORES = 8
BL = B // NCORES          # 4 batches per core
BM = BL * M               # 80 state rows per core
RPP = 32                  # (s,l) rows per partition in encode tiles
TILE_ROWS = 128 * RPP     # 4096 rows per encode tile = 64 sentences
NCHUNK = (S * L) // TILE_ROWS   # 4 chunks
SC = TILE_ROWS // L       # 64 sentences per chunk
NBUF = 3

_built = {}


def _build(apply_mult: bool, a_is_one: bool, a: float, n_steps: int = S):
    import concourse.bacc as bacc
    import concourse.tile as tile
    import concourse.mybir as mybir

    f32 = mybir.dt.float32
    fp16 = mybir.dt.float16
    Alu = mybir.AluOpType
    Act = mybir.ActivationFunctionType
    AX = mybir.AxisListType.X

    nc = bacc.Bacc("TRN2", target_bir_lowering=False, debug=False)

    x = nc.dram_tensor("x", [BL, S * L, D], f32, kind="ExternalInput").ap()
    c_ut = nc.dram_tensor("c_ut", [D, D], fp16, kind="ExternalInput").ap()
    c_wt = nc.dram_tensor("c_wt", [D, D], f32, kind="ExternalInput").ap()
    c_keystn = nc.dram_tensor("c_keystn", [D, M], f32, kind="ExternalInput").ap()
    c_kvbc = nc.dram_tensor("c_kvbc", [BM, 4, D], fp16, kind="ExternalInput").ap()
    c_mem0 = nc.dram_tensor("c_mem0", [BM, D], f32, kind="ExternalInput").ap()
    c_memt0 = nc.dram_tensor("c_memt0", [D, BM], fp16, kind="ExternalInput").ap()
    c_id80 = nc.dram_tensor("c_id80", [BM, BM], f32, kind="ExternalInput").ap()
    c_sel = nc.dram_tensor("c_sel", [128, SC], fp16, kind="ExternalInput").ap()
    c_seln = nc.dram_tensor("c_seln", [128, SC], fp16, kind="ExternalInput").ap()
    if apply_mult:
        c_pat = nc.dram_tensor("c_pat", [128, RPP, D], f32, kind="ExternalInput").ap()
    out = nc.dram_tensor("out", [BM, D], f32, kind="ExternalOutput").ap()

    with tile.TileContext(nc) as tc, ExitStack() as ctx:
        consts = ctx.enter_context(tc.tile_pool(name="consts", bufs=1))
        persist = ctx.enter_context(tc.tile_pool(name="persist", bufs=1))
        enc_in = ctx.enter_context(tc.tile_pool(name="enc_in", bufs=3))
        enc_w = ctx.enter_context(tc.tile_pool(name="enc_w", bufs=3))
        state = ctx.enter_context(tc.tile_pool(name="state", bufs=6))
        work = ctx.enter_context(tc.tile_pool(name="work", bufs=8))
        ps_enc = ctx.enter_context(tc.tile_pool(name="ps_enc", bufs=2, space="PSUM"))
        ps_enc2 = ctx.enter_context(tc.tile_pool(name="ps_enc2", bufs=2, space="PSUM"))
        ps_cand = ctx.enter_context(tc.tile_pool(name="ps_cand", bufs=2, space="PSUM"))
        ps_t = ctx.enter_context(tc.tile_pool(name="ps_t", bufs=2, space="PSUM"))

        def load_const(ap, shape, tag, dt=fp16):
            t = consts.tile(shape, dt, tag=tag)
            nc.sync.dma_start(t, ap)
            return t

        ut_sb = load_const(c_ut, [D, D], "ut")
        wt_sb = load_const(c_wt, [D, D], "wt", f32)
        keystn_sb = load_const(c_keystn, [D, M], "keystn", f32)
        kvbc_sb = load_const(c_kvbc, [BM, 4, D], "kvbc")
        id80_sb = load_const(c_id80, [BM, BM], "id80", f32)
        sel_sb = load_const(c_sel, [128, SC], "sel")
        seln_sb = load_const(c_seln, [128, SC], "seln")
        if apply_mult:
            pat_sb = load_const(c_pat, [128, RPP, D], "pat", f32)

        # per-chunk rotating scan operands
        sbc = persist.tile([BM, NBUF, SC, D], fp16)  # -enc per state row
        cws = persist.tile([BM, NBUF, SC, D], fp16)  # W s + keys V^T
        nkg = persist.tile([BM, NBUF, SC], f32)      # -key_gate
        encB = persist.tile([BL, SC, D], fp16)       # -enc, b-major
        encT = persist.tile([D, BL, 2, SC], f32)     # enc, d-major (mod-2)
        wsB = persist.tile([BL, SC, D], fp16)        # W s, b-major
        ones80 = persist.tile([BM, 1], f32)
        nc.vector.memset(ones80, 1.0)

        def enc_misc_chunk(c):
            """DMA loads, PE matmuls, ACT copies, ladder DMAs for chunk c.
            Yields between ops. Stores xt tiles for the dve gen via a list."""
            pg = c % NBUF
            pe2 = c % 2
            xts = xt_tiles[c % 2]
            for b in range(BL):
                xt = enc_in.tile([128, RPP, D], f32, tag="xt")
                qeng = (
                    [nc.gpsimd, nc.sync, nc.scalar, nc.gpsimd][b]
                    if c == 0 else nc.gpsimd
                )
                qeng.dma_start(
                    xt,
                    x[b, c * TILE_ROWS:(c + 1) * TILE_ROWS, :].rearrange(
                        "(p r) d -> p r d", p=128
                    ),
                )
                xts.append(xt)
                xt_step[c % 2][b] = cur_step[0]
                yield
            # wait for tree (dve gen) to produce reds; sbc ladders first so
            # the next chunk's first steps unblock early
            for b in range(BL):
                while len(red_tiles[c % 2]) <= b:
                    yield
                red = red_tiles[c % 2][b]
                epB = ps_enc2.tile([SC, D], f32, tag="epB")
                nc.tensor.matmul(epB, lhsT=seln_sb, rhs=red, start=True, stop=True)
                stb = enc_w.tile([SC, D], fp16, tag="stb")
                nc.scalar.copy(stb, epB)
                nc.sync.dma_start(encB[b:b + 1, :, :], stb)
                yield
                epT = ps_enc.tile([D, SC], f32, tag="epT")
                nc.tensor.matmul(epT, lhsT=red, rhs=sel_sb, start=True, stop=True)
                nc.scalar.copy(encT[:, b, pe2, :], epT)
                yield

            def ladder(dst, src_t, ts_):
                nc.sync.dma_start(dst[0:4, pg, ts_, :], src_t[:, ts_, :])
                yield
                nc.sync.dma_start(dst[4:8, pg, ts_, :], dst[0:4, pg, ts_, :])
                nc.sync.dma_start(dst[8:12, pg, ts_, :], dst[0:4, pg, ts_, :])
                nc.sync.dma_start(dst[12:16, pg, ts_, :], dst[0:4, pg, ts_, :])
                yield
                nc.sync.dma_start(dst[16:32, pg, ts_, :], dst[0:16, pg, ts_, :])
                nc.sync.dma_start(dst[32:48, pg, ts_, :], dst[0:16, pg, ts_, :])
                nc.sync.dma_start(dst[48:64, pg, ts_, :], dst[0:16, pg, ts_, :])
                nc.sync.dma_start(dst[64:80, pg, ts_, :], dst[0:16, pg, ts_, :])
                yield

            for Q in range(4):
                ts_ = slice(Q * 16, (Q + 1) * 16)
                for y in ladder(sbc, encB, ts_):
                    yield
            # ws = enc W^T, b-major
            for b in range(BL):
                for h in range(2):
                    wp = ps_enc2.tile([RPP, D], f32, tag="epB")
                    nc.tensor.matmul(
                        wp,
                        lhsT=encT[:, b, pe2, h * RPP:(h + 1) * RPP],
                        rhs=wt_sb, start=True, stop=True,
                    )
                    wb_ = enc_w.tile([RPP, D], fp16, tag="wb")
                    nc.scalar.copy(wb_, wp)
                    nc.sync.dma_start(
                        wsB[b:b + 1, h * RPP:(h + 1) * RPP, :], wb_
                    )
                    yield
            # nkg = -(keys . s) via PE: [M, SC] per b, staged then strided DMA
            for b in range(BL):
                kgp = ps_enc2.tile([M, SC], f32, tag="epB")
                nc.tensor.matmul(
                    kgp, lhsT=keystn_sb, rhs=encT[:, b, pe2, :],
                    start=True, stop=True,
                )
                kgst = enc_w.tile([M, SC], f32, tag="kgst")
                nc.scalar.copy(kgst, kgp)
                nc.sync.dma_start(nkg[b::BL, pg, :], kgst)
                yield
            for Q in range(4):
                ts_ = slice(Q * 16, (Q + 1) * 16)
                for y in ladder(cws, wsB, ts_):
                    yield
                pool_ready[c % 2][Q] = True
                yield

        def enc_dve_chunk(c):
            """word-sum tree + cws += keysV on DVE, pulled at the DVE queue
            tail each step; waits ~3 steps after the xt DMA is issued."""
            pg = c % NBUF
            xts = xt_tiles[c % 2]
            reds = red_tiles[c % 2]
            for b in range(BL):
                while len(xts) <= b:
                    yield
                while c > 0 and cur_step[0] < xt_step[c % 2].get(b, 0) + 5:
                    yield
                xt = xts[b]
                if apply_mult:
                    for q in range(4):
                        nc.vector.tensor_mul(
                            xt[:, q * 8:(q + 1) * 8, :],
                            xt[:, q * 8:(q + 1) * 8, :],
                            pat_sb[:, q * 8:(q + 1) * 8, :],
                        )
                        yield
                y32 = enc_in.tile([128, 16, D], f32, tag="y")
                for q in range(4):
                    nc.vector.tensor_add(
                        y32[:, q * 4:(q + 1) * 4, :],
                        xt[:, q * 4:(q + 1) * 4, :],
                        xt[:, 16 + q * 4:16 + (q + 1) * 4, :],
                    )
                    yield
                nc.vector.tensor_add(y32[:, 0:8, :], y32[:, 0:8, :], y32[:, 8:16, :])
                yield
                nc.vector.tensor_add(y32[:, 0:4, :], y32[:, 0:4, :], y32[:, 4:8, :])
                yield
                nc.vector.tensor_add(y32[:, 0:2, :], y32[:, 0:2, :], y32[:, 2:4, :])
                yield
                red = enc_in.tile([128, D], fp16, tag=f"red{b % 2}")
                nc.vector.tensor_add(red, y32[:, 0, :], y32[:, 1, :])
                reds.append(red)
                yield
            for Q in range(4):
                while not pool_ready[c % 2][Q]:
                    yield
                for q in range(4):
                    sl = slice(Q * 16 + q * 4, Q * 16 + (q + 1) * 4)
                    nc.vector.tensor_tensor(
                        out=cws[:, pg, sl, :], in0=cws[:, pg, sl, :],
                        in1=kvbc_sb, op=Alu.add,
                    )
                    yield
                if Q == 0:
                    q0_done[c % 2] = True

        xt_tiles = [[], []]
        red_tiles = [[], []]
        xt_step = [{}, {}]
        q0_done = [False, False]
        pool_ready = [[False] * 4, [False] * 4]
        cur_step = [0]

        # ---- prologue: encode chunk 0 fully, init state
        P = state.tile([BM, D], f32, tag="P")
        nc.sync.dma_start(P, c_mem0)
        pT = state.tile([D, BM], fp16, tag="pT")
        nc.sync.dma_start(pT, c_memt0)

        pool_ready[0] = [False] * 4
        xt_tiles[0].clear()
        red_tiles[0].clear()
        q0_done[0] = False
        g_m0 = enc_misc_chunk(0)
        g_d0 = enc_dve_chunk(0)
        gens0 = [g_m0, g_d0]
        while gens0 and not q0_done[0]:
            for g in gens0[:]:
                if next(g, "done") == "done":
                    gens0.remove(g)
        leftover0 = gens0

        alpha = work.tile([BM, 1], f32, tag="alpha")
        nc.vector.memset(alpha, 1.0)
        cand = ps_cand.tile([BM, D], f32, tag="cand")
        nc.tensor.matmul(cand, lhsT=pT, rhs=ut_sb, start=True, stop=True)
        cf = state.tile([BM, D], f32, tag="cf")
        nc.vector.tensor_tensor(
            out=cf, in0=cand, in1=cws[:, 0, 0, :], op=Alu.add
        )
        mem0 = state.tile([BM, D], f32, tag="mem")
        nc.vector.tensor_copy(mem0, P)
        if not a_is_one:
            cfp = state.tile([BM, D], f32, tag="cfp")
            nc.scalar.activation(cfp, cf, func=Act.Prelu, alpha=float(a))
            cf = cfp
        zscr = persist.tile([BM, D], f32)
        sscr = persist.tile([BM, D], f32)
        zp = work.tile([BM, 1], f32, tag="zp")
        nc.vector.scalar_tensor_tensor(
            out=zscr, in0=P, scalar=1.0, in1=sbc[:, 0, 0, :],
            op0=Alu.bypass, op1=Alu.mult, accum_out=zp,
        )
        e_t = work.tile([BM, 1], f32, tag="e")
        nc.scalar.activation(
            e_t, zp, func=Act.Exp, scale=alpha, bias=nkg[:, 0, 0:1]
        )
        mem = mem0  # alpha0 = 1

        misc_iters = []
        dve_iters = []
        for g in leftover0:
            if g is g_m0:
                misc_iters.append((0, g))
            else:
                dve_iters.append((0, g))

        def pull(iters, n):
            for _ in range(n):
                if not iters:
                    return
                if next(iters[0][1], "done") == "done":
                    iters.pop(0)

        for t in range(n_steps):
            cur_step[0] = t
            c, i = t // SC, t % SC
            t1 = min(t + 1, n_steps - 1)
            c1, i1 = t1 // SC, t1 % SC
            pg1 = c1 % NBUF
            last = t == n_steps - 1
            if i == 0:
                nxt = []
                if c == 0 and NCHUNK > 1:
                    nxt.append(1)
                if c + 2 < NCHUNK:
                    nxt.append(c + 2)
                for cn in nxt:
                    pool_ready[cn % 2] = [False] * 4
                    xt_tiles[cn % 2].clear()
                    red_tiles[cn % 2].clear()
                    misc_iters.append((cn, enc_misc_chunk(cn)))
                    dve_iters.append((cn, enc_dve_chunk(cn)))

            # ---- DVE: d = e + 1 (cycle head)
            d_t = work.tile([BM, 1], f32, tag="dd")
            nc.vector.tensor_scalar(
                out=d_t, in0=e_t, scalar1=1.0, scalar2=None, op0=Alu.add
            )
            # ---- DVE: gate, state update, dots
            g_t = work.tile([BM, 1], f32, tag="gate")
            nc.vector.reciprocal(g_t, d_t)
            P_n = state.tile([BM, D], f32, tag="P")
            nc.vector.scalar_tensor_tensor(
                out=P_n, in0=cf, scalar=g_t, in1=mem, op0=Alu.mult, op1=Alu.add
            )
            ssq = work.tile([BM, 1], f32, tag="ssq")
            nc.vector.scalar_tensor_tensor(
                out=sscr, in0=P_n, scalar=1.0, in1=P_n,
                op0=Alu.bypass, op1=Alu.mult, accum_out=ssq,
            )
            if not last:
                zp_n = work.tile([BM, 1], f32, tag="zp")
                nc.vector.scalar_tensor_tensor(
                    out=zscr, in0=P_n, scalar=1.0, in1=sbc[:, pg1, i1, :],
                    op0=Alu.bypass, op1=Alu.mult, accum_out=zp_n,
                )
            # ---- ACT: alpha chain + gate exp
            lnt = work.tile([BM, 1], f32, tag="lnt")
            nc.scalar.activation(lnt, ssq, func=Act.Ln)
            alpha_n = work.tile([BM, 1], f32, tag="alpha")
            nc.scalar.activation(alpha_n, lnt, func=Act.Exp, scale=-0.5)
            if not last:
                e_n = work.tile([BM, 1], f32, tag="e")
                nc.scalar.activation(
                    e_n, zp_n, func=Act.Exp, scale=alpha_n,
                    bias=nkg[:, pg1, i1:i1 + 1]
                )
            if not last:
                tp_ps = ps_t.tile([D, BM], f32, tag="tp")
                nc.tensor.transpose(tp_ps, P_n, id80_sb)
                pT_n = state.tile([D, BM], fp16, tag="pT")
                nc.vector.tensor_copy(pT_n, tp_ps)
                cand_n = ps_cand.tile([BM, D], f32, tag="cand")
                nc.tensor.matmul(cand_n, lhsT=pT_n, rhs=ut_sb, start=True, stop=True)
                cf_n = state.tile([BM, D], f32, tag="cf")
                nc.vector.scalar_tensor_tensor(
                    out=cf_n, in0=cand_n, scalar=alpha_n, in1=cws[:, pg1, i1, :],
                    op0=Alu.mult, op1=Alu.add,
                )
                if not a_is_one:
                    cfp = state.tile([BM, D], f32, tag="cfp")
                    nc.scalar.activation(cfp, cf_n, func=Act.Prelu, alpha=float(a))
                    cf_n = cfp
                mem_n = state.tile([BM, D], f32, tag="mem")
                nc.scalar.activation(mem_n, P_n, func=Act.Copy, scale=alpha_n)
                pull(dve_iters, 2)
                pull(misc_iters, 4)
                P, mem, cf, e_t, alpha = P_n, mem_n, cf_n, e_n, alpha_n
            else:
                P, alpha = P_n, alpha_n

        while misc_iters or dve_iters:
            pull(misc_iters, 1)
            pull(dve_iters, 1)

        out_sb = persist.tile([BM, D], f32)
        nc.vector.tensor_scalar(
            out=out_sb, in0=P, scalar1=alpha, scalar2=None, op0=Alu.mult
        )
        nc.sync.dma_start(out, out_sb)

    nc.compile()

    try:
        import concourse.mybir as mybir2
        from concourse.hw_specs import get_activation_tables

        tabs = get_activation_tables(nc.m.arch)
        names = list(tabs.keys())
        joint = names.index("natural_log_exp_and_others")
        need = {
            mybir2.ActivationFunctionType.Exp,
            mybir2.ActivationFunctionType.Ln,
            mybir2.ActivationFunctionType.Copy,
        }
        if not a_is_one:
            need.add(mybir2.ActivationFunctionType.Prelu)
        joint_funcs = tabs[names[joint]]
        if need <= joint_funcs:
            for fn in nc.m.functions:
                for blk in fn.blocks:
                    newinsts = []
                    loaded = False
                    for inst in blk.instructions:
                        if isinstance(inst, mybir2.InstLoadActFuncSet):
                            if not loaded:
                                inst.act_func_set_id = joint
                                loaded = True
                                newinsts.append(inst)
                            continue
                        newinsts.append(inst)
                    blk.instructions[:] = newinsts
    except Exception:
        pass

    return nc


def _perm():
    # state row r = m*BL + b  <->  reference row b*M + m
    p = np.empty(BM, dtype=np.int64)
    for m_ in range(M):
        for b_ in range(BL):
            p[m_ * BL + b_] = b_ * M + m_
    return p


def _consts(enc_mult, keys, U, V, W, apply_mult):
    f = np.float32
    h = np.float16
    keys = np.asarray(keys, f)
    U = np.asarray(U, f)
    V = np.asarray(V, f)
    W = np.asarray(W, f)
    enc_mult = np.asarray(enc_mult, f)

    sel = np.zeros((128, SC), f)
    for p in range(128):
        sel[p, p // (L // RPP)] = 1.0

    keysv = keys @ V.T          # [M, D]
    perm = _perm()              # state row -> (b*M+m)
    keys_tiled = np.tile(keys, (BL, 1))[perm]      # row m*4+b = keys[m]
    keysv_tiled = np.tile(keysv, (BL, 1))[perm]
    mem0 = np.tile(keys, (BL, 1))[perm]

    c = {
        "c_ut": np.ascontiguousarray(U.T).astype(h),
        "c_wt": np.ascontiguousarray(W.T).astype(f),
        "c_keystn": np.ascontiguousarray(-keys.T).astype(f),
        "c_kvbc": np.ascontiguousarray(
            np.repeat(keysv_tiled[:, None, :], 4, axis=1)).astype(h),
        "c_mem0": np.ascontiguousarray(mem0).astype(f),
        "c_memt0": np.ascontiguousarray(mem0.T).astype(h),
        "c_id80": np.eye(BM, dtype=f),
        "c_sel": sel.astype(h),
        "c_seln": (-sel).astype(h),
    }
    if apply_mult:
        pat = np.empty((128, RPP, D), f)
        for p in range(128):
            for r in range(RPP):
                pat[p, r, :] = enc_mult[(p * RPP + r) % L, :]
        c["c_pat"] = pat
    return c


def kernel(batch, enc_mult, keys, U, V, W, prelu_a):
    from concourse.bass_utils import run_bass_kernel_spmd

    batch = np.asarray(batch, np.float32)
    enc_mult = np.asarray(enc_mult, np.float32)
    a = float(np.asarray(prelu_a))
    apply_mult = not bool(np.all(enc_mult == 1.0))
    a_is_one = a == 1.0

    key = (apply_mult, a_is_one, a)
    if key not in _built:
        _built[key] = _build(apply_mult, a_is_one, a)
    nc = _built[key]

    consts = _consts(enc_mult, keys, U, V, W, apply_mult)
    in_maps = []
    for cidx in range(NCORES):
        m = dict(consts)
        m["x"] = np.ascontiguousarray(
            batch[cidx * BL:(cidx + 1) * BL].reshape(BL, S * L, D)
        )
        in_maps.append(m)

    trace = os.environ.get("ENTNET_TRACE", "") == "1"
    res = run_bass_kernel_spmd(
        nc, in_maps, core_ids=list(range(NCORES)), trace=trace
    )
    if trace:
        print(f"HW exec time: {res.exec_time_ns} ns")
        if res.instructions_and_trace is not None:
            print(f"trace: {res.instructions_and_trace[1]}")

    perm = _perm()
    inv = np.empty_like(perm)
    inv[perm] = np.arange(BM)
    return np.concatenate(
        [r["out"][inv].reshape(BL, M, D) for r in res.results], axis=0
    )
